# revision 1
# baseline (speedup 1.0000x reference)
"""Trainium2 Bass kernel: causal self-attention with RoPE.

Problem: B=2, T=2048, C=1536, H=16 heads, D=96 head dim.
  qkv = x @ w_attn.T ; rope(q, k) ; causal softmax attention ; y = att @ w_proj.T

Sharding (8 cores): data-parallel over batch (2) x tensor-parallel over heads
(4 groups of 4 heads).  Each core computes, for its batch b and its 4 heads:
  - QKV projection (its heads' slice of w_attn),
  - RoPE + causal flash-style attention,
  - partial output projection y_part = att_heads @ w_proj[:, cols].T
The 4 partial [T, C] outputs per batch are summed on the host (row-split
contraction => reduce at the host boundary, which is where unsharding happens).

All input transposes (x^T, w^T) and the RoPE cos/sin tables are precomputed on
the host in numpy and passed as extra DRAM inputs; the device never transposes
inputs.  The RoPE even/odd d-permutation is folded into the weight row order so
on-device rope is pure contiguous elementwise ops.

Matmuls run as float32r (full-rate fp32 path on trn2 PE for free dim >= 256).
"""

import math

import numpy as np

import concourse.bass as bass
import concourse.mybir as mybir
import concourse.tile as tile
from concourse import bacc, bass_utils
from concourse.masks import make_identity

# ---------------------------------------------------------------- constants
B, T, C = 2, 2048, 1536
H, D = 16, 96
NCORES = 8
HPC = 4                      # heads per core
DH = HPC * D                 # 384 = per-core head-dim total
DH2 = HPC * (D // 2)         # 192 = per-core evens (or odds) width
SCALE = 1.0 / math.sqrt(D)
NT = T // 128                # 16 t-tiles of 128 tokens
NQ = T // 512                # 4 q-tiles of 512 queries
F32 = mybir.dt.float32
F32R = mybir.dt.float32r

MASK_ENG = lambda nc: nc.vector  # engine for causal-mask muls (A/B-able)
PHASES = "AB"               # which phases to emit (cost-model experiments)
TRN_DT = F32R               # dtype of rope->transpose chain
PPQ_BUFS = 2                # phase A qkv psum double-buffering
PART_BUFS = 2               # rope temp tile bufs
PAX_BUFS = 14               # x tile pool bufs
PST_BUFS = 2                # phase B S^T psum double-buffering
ACC_BUFS = 2                # PV accumulator psum bufs
OP_BUFS = 1                 # projection psum bufs
WP_PRELOAD = False          # load wp_sb at kernel start (persist pool)
PROJ_PACK = False           # repack attT -> [128, 3, 512] so proj contracts K=128

# Matmul operands are float32r: full-rate fp32 path on trn2 PE (free dim
# >= 256).  The BIR verifier requires fp32r matmul inputs to come from a
# rounding producer: compute-engine writes with out dtype f32r, or a gpsimd
# (SWDGE) DMA into an f32r tile -- both verified bit-identical on HW.


# ---------------------------------------------------------------- device IR
def _build_kernel(reps=1):
    nc = bacc.Bacc(
        "TRN2",
        target_bir_lowering=False,
        debug=False,
        enable_asserts=False,
        num_devices=NCORES,
    )

    xT = nc.dram_tensor("xT", [C, T], F32R, kind="ExternalInput").ap()
    wqkvT = nc.dram_tensor("wqkvT", [C, 3 * DH], F32R, kind="ExternalInput").ap()
    wpT_shape = [128, 3, C] if PROJ_PACK else [D, HPC, C]
    wpT = nc.dram_tensor("wpT", wpT_shape, F32R, kind="ExternalInput").ap()
    ctab = nc.dram_tensor("ctab", [T, DH2], F32, kind="ExternalInput").ap()
    stab = nc.dram_tensor("stab", [T, DH2], F32, kind="ExternalInput").ap()
    tmd = nc.dram_tensor("tm", [128, 1024], F32, kind="ExternalInput").ap()
    yp = nc.dram_tensor("yp", [T, C], F32, kind="ExternalOutput").ap()

    with tile.TileContext(nc) as tc:
        for _ in range(reps):
            _body(tc, xT, wqkvT, wpT, ctab, stab, tmd, yp)

    nc.compile()
    return nc


def _body(tc, xT, wqkvT, wpT, ctab, stab, tmd, yp):
    nc = tc.nc
    Exp = mybir.ActivationFunctionType.Exp

    with tc.tile_pool(name="persist", bufs=1) as persist:
        # Rope'd transposed Q/K: [d, head, t]; d-order = rope output order.
        QT = persist.tile([D, HPC, T], F32R)
        KT = persist.tile([D, HPC, T], F32R)
        # V in sequence-major layout per k-block, with a ones column (97th)
        # so the PV matmul also produces the softmax denominators.
        V = persist.tile([128, HPC, NT, D + 1], F32R)
        tm = persist.tile([128, 1024], F32)
        ones1 = persist.tile([1, D], F32R)

        wp_pre = None
        if WP_PRELOAD:
            wp_pre = persist.tile([D, HPC, C], F32R, name="wp_pre")
            nc.sync.dma_start(out=wp_pre, in_=wpT)
        # memset can't write f32r directly; memset f32 then round via copy
        onesf = persist.tile([128, D], F32)
        nc.vector.memset(onesf, 1.0)
        nc.scalar.copy(
            out=V[:, :, :, D],
            in_=onesf[:, 0 : HPC * NT].rearrange("p (h t) -> p h t", h=HPC),
        )
        nc.scalar.copy(out=ones1, in_=onesf[0:1, :])

        # ---------------- Phase A: QKV projection + rope + Q/K transpose ----
        with (
            tc.tile_pool(name="pa", bufs=1) as pa,
            tc.tile_pool(name="pax", bufs=PAX_BUFS) as pax,
            tc.tile_pool(name="par", bufs=2) as par,
            tc.tile_pool(name="ptab", bufs=2) as ptab,
            tc.tile_pool(name="ppq", bufs=PPQ_BUFS, space="PSUM") as ppq,
            tc.tile_pool(name="pptr", bufs=1, space="PSUM") as pptr,
        ):
            wq_sb = pa.tile([128, 12, 3 * DH], F32R)
            ident = pa.tile([128, 128], TRN_DT)
            identf = pa.tile([128, 128], F32)
            # Interleave the first quarter's x tiles (sync HWDGE ring) with
            # the weight tiles (scalar HWDGE ring): both rings are FIFO, so
            # this ordering lets the first matmul start after ~one tile of
            # each instead of after the whole weight load.
            xcs0 = []
            for c in range(12):
                xc = pax.tile([128, 512], F32R, name=f"xc0_{c}", tag="xc")
                nc.sync.dma_start(out=xc, in_=xT[c * 128 : (c + 1) * 128, 0:512])
                nc.scalar.dma_start(
                    out=wq_sb[:, c, :],
                    in_=wqkvT[c * 128 : (c + 1) * 128, :],
                )
                xcs0.append(xc)
            make_identity(nc, identf)
            nc.scalar.copy(out=ident, in_=identf)

            for tq in range(4):  # 512-token quarters
                ts0 = tq * 512
                ct = ptab.tile([128, 4, DH2], F32, tag="ct")
                st = ptab.tile([128, 4, DH2], F32, tag="st")
                nc.sync.dma_start(
                    out=ct,
                    in_=ctab[ts0 : ts0 + 512, :].rearrange(
                        "(tt p) d -> p tt d", p=128
                    ),
                )
                nc.sync.dma_start(
                    out=st,
                    in_=stab[ts0 : ts0 + 512, :].rearrange(
                        "(tt p) d -> p tt d", p=128
                    ),
                )
                if tq == 0:
                    xcs = xcs0
                else:
                    xcs = []
                    for c in range(12):
                        xc = pax.tile(
                            [128, 512], F32R, name=f"xc{tq}_{c}", tag="xc"
                        )
                        nc.sync.dma_start(
                            out=xc,
                            in_=xT[c * 128 : (c + 1) * 128, ts0 : ts0 + 512],
                        )
                        xcs.append(xc)

                for tt in range(4):
                    t0 = tq * 4 + tt  # global 128-token tile index
                    qp = ppq.tile([128, DH], F32, tag="qp")
                    kp = ppq.tile([128, DH], F32, tag="kp")
                    vp = ppq.tile([128, DH], F32, tag="vp")
                    for c in range(12):
                        lhs = xcs[c][:, tt * 128 : (tt + 1) * 128]
                        w = wq_sb[:, c, :]
                        s0 = c == 0
                        s1 = c == 11
                        nc.tensor.matmul(
                            qp, lhs, w[:, 0:DH], start=s0, stop=s1
                        )
                        nc.tensor.matmul(
                            kp, lhs, w[:, DH : 2 * DH], start=s0, stop=s1
                        )
                        nc.tensor.matmul(
                            vp, lhs, w[:, 2 * DH : 3 * DH], start=s0, stop=s1
                        )

                    # V: [t, (h d)] -> V[:, h, t0, 0:D]
                    nc.scalar.copy(
                        out=V[:, :, t0, 0:D],
                        in_=vp.rearrange("p (h d) -> p h d", h=HPC),
                    )

                    ctt = ct[:, tt, :]
                    stt = st[:, tt, :]
                    qr = par.tile([128, HPC, 2, D // 2], TRN_DT, tag="qr")
                    kr = par.tile([128, HPC, 2, D // 2], TRN_DT, tag="kr")
                    for (src, dst, tag) in ((qp, qr, "q"), (kp, kr, "k")):
                        e = src[:, 0:DH2]
                        o = src[:, DH2:DH]
                        t1 = par.tile([128, DH2], F32, tag=f"t1{tag}", bufs=PART_BUFS)
                        t2 = par.tile([128, DH2], F32, tag=f"t2{tag}", bufs=PART_BUFS)
                        t3 = par.tile([128, DH2], F32, tag=f"t3{tag}", bufs=PART_BUFS)
                        t4 = par.tile([128, DH2], F32, tag=f"t4{tag}", bufs=PART_BUFS)
                        nc.vector.tensor_mul(t1, e, ctt)
                        nc.vector.tensor_mul(t2, o, stt)
                        nc.vector.tensor_mul(t3, e, stt)
                        nc.vector.tensor_mul(t4, o, ctt)
                        nc.vector.tensor_sub(
                            dst[:, :, 0, :],
                            t1.rearrange("p (h d) -> p h d", h=HPC),
                            t2.rearrange("p (h d) -> p h d", h=HPC),
                        )
                        nc.vector.tensor_add(
                            dst[:, :, 1, :],
                            t3.rearrange("p (h d) -> p h d", h=HPC),
                            t4.rearrange("p (h d) -> p h d", h=HPC),
                        )

                    # transpose rope'd q/k tiles: [128t, 96d] -> [96d, 128t]
                    tpq = pptr.tile([D, HPC, 128], TRN_DT, tag="tpq")
                    tpk = pptr.tile([D, HPC, 128], TRN_DT, tag="tpk")
                    for h in range(HPC):
                        nc.tensor.transpose(tpq[:, h], qr[:, h], ident)
                        nc.tensor.transpose(tpk[:, h], kr[:, h], ident)
                    nc.scalar.copy(
                        out=QT[:, :, t0 * 128 : (t0 + 1) * 128], in_=tpq
                    )
                    nc.vector.tensor_copy(
                        KT[:, :, t0 * 128 : (t0 + 1) * 128], tpk
                    )

        # ---------------- Phase B: attention + output projection ------------
        if "B" not in PHASES:
            return
        with (
            tc.tile_pool(name="pb", bufs=1) as pb,
            tc.tile_pool(name="pbt", bufs=2) as pbt,
            tc.tile_pool(name="ppt", bufs=3) as ppt,
            tc.tile_pool(name="pbr", bufs=2) as pbr,
            tc.tile_pool(name="pst", bufs=PST_BUFS, space="PSUM") as pst,
            tc.tile_pool(name="pacc", bufs=ACC_BUFS, space="PSUM") as pacc,
            tc.tile_pool(name="prepop", bufs=2, space="PSUM") as prepop,
        ):
            nc.sync.dma_start(out=tm, in_=tmd)
            if wp_pre is not None:
                wp_sb = wp_pre
            elif PROJ_PACK:
                wp_sb = pb.tile([128, 3, C], F32R)
                nc.sync.dma_start(out=wp_sb, in_=wpT)
            else:
                wp_sb = pb.tile([D, HPC, C], F32R)
                nc.sync.dma_start(out=wp_sb, in_=wpT)

            for i in reversed(range(NQ)):  # big q-tiles first
                q0 = i * 512
                attT = pbt.tile([D, HPC, 512], F32R, tag="attT")
                for h in range(HPC):
                    acc = pacc.tile([D + 1, 512], F32, tag="acc")
                    jmax = 4 * i + 3
                    for jp in range(2 * i + 2):
                        stp = pst.tile([128, 2, 512], F32, tag="stp")
                        for jj in range(2):
                            j = 2 * jp + jj
                            nc.tensor.matmul(
                                stp[:, jj],
                                KT[:, h, j * 128 : (j + 1) * 128],
                                QT[:, h, q0 : q0 + 512],
                                start=True,
                                stop=True,
                            )
                        pt = ppt.tile([128, 2, 512], F32R, tag="pt")
                        nc.scalar.activation(
                            pt.rearrange("p a b -> p (a b)"),
                            stp.rearrange("p a b -> p (a b)"),
                            Exp,
                            scale=SCALE,
                        )
                        for jj in range(2):
                            j = 2 * jp + jj
                            if j >= 4 * i:  # diagonal block: causal mask
                                off = j * 128 - q0
                                w = off + 128
                                MASK_ENG(nc).tensor_mul(
                                    pt[:, jj, 0:w],
                                    pt[:, jj, 0:w],
                                    tm[:, 512 - off : 512 - off + w],
                                )
                        for jj in range(2):
                            j = 2 * jp + jj
                            nc.tensor.matmul(
                                acc,
                                V[:, h, j],
                                pt[:, jj],
                                start=(j == 0),
                                stop=(j == jmax),
                            )
                    # normalize: attT[:, h] = acc[0:D] * (1 / acc[D]) per col
                    r1 = pbr.tile([1, 512], F32R, tag="r1")
                    with nc.allow_low_precision(reason="f32r recip, 5e-4 rel"):
                        nc.vector.reciprocal(r1, acc[D : D + 1, :])
                    rep = prepop.tile([D, 512], F32, tag="ro", padded_shape=[128, 512])
                    nc.tensor.matmul(
                        rep, ones1, r1, start=True, stop=True
                    )
                    reps = pbr.tile([D, 512], F32, tag="reps")
                    nc.vector.tensor_copy(reps, rep)
                    nc.vector.tensor_mul(attT[:, h], acc[0:D, :], reps)

                if PROJ_PACK:
                    # repack (h, d) rows into 3 x 128-partition tiles so the
                    # projection contracts K=128 per matmul (3 passes not 4)
                    attTp = pbt.tile([128, 3, 512], F32R, tag="attTp")
                    for h in range(HPC):
                        i0 = D * h
                        while i0 < D * (h + 1):
                            k, p0 = divmod(i0, 128)
                            n = min(D * (h + 1) - i0, 128 - p0)
                            nc.sync.dma_start(
                                out=attTp[p0 : p0 + n, k, :],
                                in_=attT[i0 - D * h : i0 - D * h + n, h, :],
                            )
                            i0 += n

                # output projection for this q-tile
                for tt in range(4):
                    r0 = q0 + tt * 128
                    ysb = pbr.tile([128, C], F32, tag="ysb")
                    for os in range(3):
                        op = prepop.tile([128, 512], F32, tag="ro")
                        if PROJ_PACK:
                            for kk in range(3):
                                nc.tensor.matmul(
                                    op,
                                    attTp[:, kk, tt * 128 : (tt + 1) * 128],
                                    wp_sb[:, kk, os * 512 : (os + 1) * 512],
                                    start=(kk == 0),
                                    stop=(kk == 2),
                                )
                        else:
                            for h in range(HPC):
                                nc.tensor.matmul(
                                    op,
                                    attT[:, h, tt * 128 : (tt + 1) * 128],
                                    wp_sb[:, h, os * 512 : (os + 1) * 512],
                                    start=(h == 0),
                                    stop=(h == HPC - 1),
                                )
                        nc.vector.tensor_copy(
                            ysb[:, os * 512 : (os + 1) * 512], op
                        )
                    nc.sync.dma_start(out=yp[r0 : r0 + 128, :], in_=ysb)


# ---------------------------------------------------------------- host side
def _rope_tables():
    inv_freq = 1.0 / (10000.0 ** (np.arange(0, D, 2, dtype=np.float32) / D))
    t = np.arange(T, dtype=np.float32)
    freqs = np.outer(t, inv_freq)                       # [T, 48]
    emb = np.concatenate([freqs, freqs], axis=-1)       # [T, 96]
    c = np.cos(emb)[:, ::2].astype(np.float32)          # [T, 48]
    s = np.sin(emb)[:, ::2].astype(np.float32)
    ctab = np.ascontiguousarray(np.tile(c, (1, HPC)))   # [T, 192]
    stab = np.ascontiguousarray(np.tile(s, (1, HPC)))
    return ctab, stab


def _tri_mask():
    # tm[k, c] = 1.0 iff c >= k + 512
    k = np.arange(128)[:, None]
    c = np.arange(1024)[None, :]
    return (c >= k + 512).astype(np.float32)


def _core_inputs(x, w_attn, w_proj, core):
    b, g = divmod(core, HPC)
    heads = [HPC * g + hh for hh in range(HPC)]
    xTh = np.ascontiguousarray(x[b].T)                  # [C, T]

    def rows(sec, h):
        return w_attn[sec * C + h * D : sec * C + (h + 1) * D]

    q_e = np.concatenate([rows(0, h)[0::2] for h in heads])   # [192, C]
    q_o = np.concatenate([rows(0, h)[1::2] for h in heads])
    k_e = np.concatenate([rows(1, h)[0::2] for h in heads])
    k_o = np.concatenate([rows(1, h)[1::2] for h in heads])
    v_r = np.concatenate([rows(2, h) for h in heads])         # [384, C]
    wqkv = np.concatenate([q_e, q_o, k_e, k_o, v_r])          # [1152, C]
    wqkvT = np.ascontiguousarray(wqkv.T)                      # [C, 1152]

    wp_flat = np.concatenate(
        [w_proj[:, h * D : (h + 1) * D].T for h in heads]
    )                                                         # [384, C], (h,d)-major
    if PROJ_PACK:
        wpT = np.ascontiguousarray(wp_flat.reshape(3, 128, C).transpose(1, 0, 2))
    else:
        wpT = np.ascontiguousarray(
            wp_flat.reshape(HPC, D, C).transpose(1, 0, 2)
        )                                                     # [96, 4, C]
    return {"xT": xTh, "wqkvT": wqkvT, "wpT": wpT}


_NC_CACHE = {}


def _get_nc(reps=1):
    if reps not in _NC_CACHE:
        _NC_CACHE[reps] = _build_kernel(reps)
    return _NC_CACHE[reps]


def make_in_maps(x, w_attn, w_proj):
    x = np.asarray(x, np.float32)
    w_attn = np.asarray(w_attn, np.float32)
    w_proj = np.asarray(w_proj, np.float32)
    ctab, stab = _rope_tables()
    tm = _tri_mask()
    in_maps = []
    for core in range(NCORES):
        m = _core_inputs(x, w_attn, w_proj, core)
        m["ctab"] = ctab
        m["stab"] = stab
        m["tm"] = tm
        in_maps.append(m)
    return in_maps


def combine_outputs(results):
    y = np.empty((B, T, C), np.float32)
    for b in range(B):
        parts = [results[b * HPC + g]["yp"] for g in range(HPC)]
        y[b] = parts[0] + parts[1] + parts[2] + parts[3]
    return y


def kernel(x, w_attn, w_proj, _trace=False, _trace_kwargs=None):
    nc = _get_nc()
    in_maps = make_in_maps(x, w_attn, w_proj)
    res = bass_utils.run_bass_kernel_spmd(
        nc,
        in_maps,
        core_ids=list(range(NCORES)),
        trace=_trace,
        **(_trace_kwargs or {}),
    )
    out = combine_outputs(res.results)
    if _trace:
        kernel._last_results = res
    return out

